# revision 27
# baseline (speedup 1.0000x reference)
# Trainium2 Bass kernel for nn_DASSM (DCN-gated selective-scan module).
#
# Sharding: 8 cores = 4 samples x 2 horizontal bands of 64 rows. All stages
# run band-local (convs/DCN use halo rows recomputed per core); the only
# cross-core dependency is the selective-scan carry at the band boundary,
# exchanged with a pair-wise AllReduce and applied as a decay-weighted
# correction folded into y (y += (cumprod(dA) * C) * h_in).
#
# Layout: channels (128) on partitions, pixels on the free dim.
# Engine plan: PE does all matmuls in bf16 (stage1 conv, depthwise conv as
# diagonal matmuls, DCN weight expansion, scan projections, LN sums);
# ACT does activations + PSUM->SBUF casts with per-partition bias; DVE and
# Pool split the DCN 9-tap apply.  dA = exp(-softplus(z)) = sigmoid(-z)
# (A = -1 exactly since D_STATE=1), so the scan prep is 2 ACT ops.
import contextlib

import ml_dtypes
import numpy as np

import concourse.bacc as bacc
import concourse.mybir as mybir
import concourse.tile as tile
from concourse.bass_utils import run_bass_kernel_spmd

F32 = mybir.dt.float32
BF16 = mybir.dt.bfloat16
AF = mybir.ActivationFunctionType
OP = mybir.AluOpType

B, C, H, W = 4, 128, 128, 128
G, GC = 8, 16
BAND = 64
XH = 3                      # halo rows of x on each side of the band
NRX = BAND + 2 * XH         # 70 rows in x_pad
NRC = BAND + 4              # 68 rows in xc_pad (band +/- 2)
WP = W + 2                  # padded width
NPIX = BAND * W             # 8192 band pixels
EPS = 1e-6


def _mm(nc, out, lhsT, rhs, start=True, stop=True):
    nc.tensor.matmul(out, lhsT, rhs, start=start, stop=stop)


def build_program():
    nc = bacc.Bacc("TRN2", target_bir_lowering=False, debug=False, num_devices=8)

    def inp(name, shape, dt=F32):
        return nc.dram_tensor(name, shape, dt, kind="ExternalInput").ap()

    x_pad = inp("x_pad", [C, NRX, WP], BF16)
    w_s1 = inp("w_s1", [C, 9 * C], BF16)    # fused in_proj*conv2d taps, lhsT
    conv2d_b = inp("conv2d_b", [C, 1])
    dw_diag = inp("dw_diag", [C, 9 * C], BF16)  # diag(dw_k tap) blocks, lhsT
    dw_b = inp("dw_b", [C, 1])
    ln1_g = inp("ln1_g", [C, 1])
    ln1_b = inp("ln1_b", [C, 1])
    off_w_lhsT = inp("off_w_lhsT", [C, 16], BF16)
    off_b_col = inp("off_b_col", [16, 1])
    e6 = inp("e6", [16, 4 * 72], BF16)      # expand one-hots (ya,yb,xa,xb)
    fybias = inp("fybias", [72, 1])         # +1 at dy=0 rows (FY const term)
    fxbias = inp("fxbias", [72, 1])         # +1 at dx=0 rows (FX const term)
    e16 = inp("e16", [72, 9 * C], BF16)     # m rows -> per-tap channel bcast
    dtW_lhsT = inp("dtW_lhsT", [C, C], BF16)    # (dt_w @ x_proj[0:8]).T
    pbWn_lhsT = inp("pbWn_lhsT", [C, C], BF16)  # -x_proj row 8 bcast
    pcW_lhsT = inp("pcW_lhsT", [C, C], BF16)    # x_proj row 9 bcast
    ndt_b_col = inp("ndt_b_col", [C, 1])    # -dt_b
    ds_col = inp("ds_col", [C, 1])
    onesb = inp("onesb", [C, C], BF16)      # 1/C everywhere (LN mean matmul)
    out_w_lhsT = inp("out_w_lhsT", [C, C], BF16)  # (out_proj_w * ln2_g).T
    wb2_col = inp("wb2_col", [C, 1])        # out_proj_w @ ln2_b
    mask_contrib = inp("mask_contrib", [C, 1])
    mask_use = inp("mask_use", [C, 1])

    out_band = nc.dram_tensor("out", [C, BAND, W], F32, kind="ExternalOutput").ap()

    with tile.TileContext(nc) as tc:
        est = contextlib.ExitStack()
        sing = est.enter_context(tc.tile_pool(name="sing", bufs=1))

        _dmae = [nc.sync, nc.scalar]

        def load(ap_dram, shape, tagname, dt=F32):
            t = sing.tile(shape, dt, tag=tagname)
            eng = _dmae[load.i % len(_dmae)]
            load.i += 1
            eng.dma_start(out=t[:], in_=ap_dram)
            return t
        load.i = 0

        s_ws1 = load(w_s1, [C, 9 * C], "s_ws1", BF16)
        s_c2b = load(conv2d_b, [C, 1], "s_c2b")
        s_dwd = load(dw_diag, [C, 9 * C], "s_dwd", BF16)
        s_dwb = load(dw_b, [C, 1], "s_dwb")
        s_l1g = load(ln1_g, [C, 1], "s_l1g")
        s_l1b = load(ln1_b, [C, 1], "s_l1b")
        s_offw = load(off_w_lhsT, [C, 16], "s_offw", BF16)
        s_offb = load(off_b_col, [16, 1], "s_offb")
        s_e6 = load(e6, [16, 4 * 72], "s_e6", BF16)
        s_fyb = load(fybias, [72, 1], "s_fyb")
        s_fxb = load(fxbias, [72, 1], "s_fxb")
        s_e16 = load(e16, [72, 9 * C], "s_e16", BF16)
        s_dtW = load(dtW_lhsT, [C, C], "s_dtW", BF16)
        s_pbWn = load(pbWn_lhsT, [C, C], "s_pbWn", BF16)
        s_pcW = load(pcW_lhsT, [C, C], "s_pcW", BF16)
        s_ndtb = load(ndt_b_col, [C, 1], "s_ndtb")
        s_ds = load(ds_col, [C, 1], "s_ds")
        s_onesb = load(onesb, [C, C], "s_onesb", BF16)
        s_outw = load(out_w_lhsT, [C, C], "s_outw", BF16)
        s_wb2 = load(wb2_col, [C, 1], "s_wb2")
        s_mc = load(mask_contrib, [C, 1], "s_mc")
        s_mu = load(mask_use, [C, 1], "s_mu")
        s_zero = sing.tile([C, 1], F32, tag="s_zero")
        nc.vector.memset(s_zero[:], 0.0)
        s_epsT = sing.tile([1, C], BF16, tag="s_epsT")   # k=1 lhsT: +eps
        nc.vector.memset(s_epsT[:], EPS)
        s_onerow = sing.tile([1, 512], BF16, tag="s_onerow")
        nc.vector.memset(s_onerow[:], 1.0)

        # ---- pool stack (LIFO): pxd > pxc > pmf > poffs > transients ----
        pxd_cm = tc.tile_pool(name="pxd", bufs=1)
        pxd = pxd_cm.__enter__()
        pxc_cm = tc.tile_pool(name="pxc", bufs=1)
        pxc = pxc_cm.__enter__()
        pma_cm = tc.tile_pool(name="pma", bufs=1)
        pma = pma_cm.__enter__()
        pmf_cm = tc.tile_pool(name="pmf", bufs=1)
        pmf = pmf_cm.__enter__()
        poffs_cm = tc.tile_pool(name="poffs", bufs=1)
        poffs = poffs_cm.__enter__()
        xc_pad = pxc.tile([C, NRC, WP], BF16)
        # only the left/right halo columns need zeroing; stage 1 writes the rest
        nc.gpsimd.memset(xc_pad[:, :, 0:1], 0.0)
        nc.gpsimd.memset(xc_pad[:, :, W + 1:W + 2], 0.0)

        # ================= stage 1: fused in_proj + conv2d + SiLU ========
        pxp_cm = tc.tile_pool(name="pxp", bufs=1)
        pxp = pxp_cm.__enter__()
        xp = pxp.tile([C, NRX, WP], BF16)
        nc.sync.dma_start(out=xp[:], in_=x_pad)
        with tc.tile_pool(name="ps1", bufs=2, space="PSUM") as ps1:
            for j0 in range(0, NRC, 4):          # 17 chunks of 4 rows
                pt = ps1.tile([C, 4 * W], F32, tag="ps1")
                for ti in range(9):
                    dy, dx = ti // 3, ti % 3
                    rhs = xp[:, j0 + dy:j0 + dy + 4, dx:dx + W]
                    _mm(nc, pt[:], s_ws1[:, ti * C:(ti + 1) * C], rhs,
                        start=(ti == 0), stop=(ti == 8))
                nc.scalar.activation(
                    out=xc_pad[:, j0:j0 + 4, 1:W + 1],
                    in_=pt[:].rearrange("p (a b) -> p a b", a=4),
                    func=AF.Silu, bias=s_c2b[:], scale=1.0)
        pxp_cm.__exit__(None, None, None)

        # ============ stage 2: depthwise conv on PE (diag matmuls) =======
        px1_cm = tc.tile_pool(name="px1", bufs=1)
        px1 = px1_cm.__enter__()
        x1 = px1.tile([C, BAND, W], BF16)
        with tc.tile_pool(name="ps2", bufs=2, space="PSUM") as ps2:
            for j0 in range(0, BAND, 4):         # 16 chunks of 4 rows
                pt = ps2.tile([C, 4 * W], F32, tag="ps2")
                for ti in range(9):
                    dy, dx = ti // 3, ti % 3
                    rhs = xc_pad[:, 1 + j0 + dy:1 + j0 + dy + 4, dx:dx + W]
                    _mm(nc, pt[:], s_dwd[:, ti * C:(ti + 1) * C], rhs,
                        start=(ti == 0), stop=(ti == 8))
                nc.scalar.activation(
                    out=x1[:, j0:j0 + 4, :],
                    in_=pt[:].rearrange("p (a b) -> p a b", a=4),
                    func=AF.Identity, bias=s_dwb[:], scale=1.0)

        # ============ LN1 (over channels) + GELU + offset proj ===========
        offs = poffs.tile([16, NPIX], BF16)
        LNC = 512
        x1f = x1[:].rearrange("p a b -> p (a b)")
        with tc.tile_pool(name="ln1t", bufs=3) as lnt, \
                tc.tile_pool(name="ln1p", bufs=2, space="PSUM") as lnp:
            for c0 in range(0, NPIX, LNC):
                xc1 = x1f[:, c0:c0 + LNC]
                sq = lnt.tile([C, LNC], BF16, tag="sq")
                nc.gpsimd.tensor_tensor(out=sq[:], in0=xc1, in1=xc1, op=OP.mult)
                pA = lnp.tile([C, LNC], F32, tag="pA")
                pB = lnp.tile([C, LNC], F32, tag="pB")
                for s0 in range(0, LNC, 512):
                    _mm(nc, pA[:, s0:s0 + 512], s_onesb[:], xc1[:, s0:s0 + 512])
                    _mm(nc, pB[:, s0:s0 + 512], s_onesb[:], sq[:, s0:s0 + 512],
                        stop=False)
                    _mm(nc, pB[:, s0:s0 + 512], s_epsT[:], s_onerow[:],
                        start=False)
                mu2 = lnt.tile([C, LNC], F32, tag="mu2")
                nc.scalar.activation(out=mu2[:], in_=pA[:], func=AF.Square,
                                     bias=s_zero[:], scale=1.0)
                varq = lnt.tile([C, LNC], F32, tag="varq")
                nc.vector.tensor_tensor(out=varq[:], in0=pB[:], in1=mu2[:],
                                        op=OP.subtract)
                vr = lnt.tile([C, LNC], F32, tag="vr")
                nc.vector.reciprocal(out=vr[:], in_=varq[:])
                r = lnt.tile([C, LNC], BF16, tag="r")
                nc.scalar.activation(out=r[:], in_=vr[:], func=AF.Sqrt,
                                     bias=s_zero[:], scale=1.0)
                xm = lnt.tile([C, LNC], BF16, tag="xm")
                nc.vector.tensor_tensor(out=xm[:], in0=xc1, in1=pA[:],
                                        op=OP.subtract)
                nc.vector.tensor_tensor(out=xc1, in0=xm[:], in1=r[:], op=OP.mult)
            for c0 in range(0, NPIX, LNC):       # batched GELU
                xc1 = x1f[:, c0:c0 + LNC]
                nc.scalar.activation(out=xc1, in_=xc1, func=AF.Gelu,
                                     bias=s_l1b[:], scale=s_l1g[:])
        with tc.tile_pool(name="offp", bufs=2, space="PSUM") as offp:
            for c0 in range(0, NPIX, LNC):       # offset projection
                po = offp.tile([16, LNC], F32, tag="po")
                for s0 in range(0, LNC, 512):
                    _mm(nc, po[:, s0:s0 + 512], s_offw[:],
                        x1f[:, c0 + s0:c0 + s0 + 512])
                nc.scalar.activation(out=offs[:, c0:c0 + LNC], in_=po[:],
                                     func=AF.Identity, bias=s_offb[:], scale=1.0)
        px1_cm.__exit__(None, None, None)

        # ================= DCN factors ===================================
        # fct[:, 0, :] = f_-1 (s then s-a); fct[:, 1, :] = f_+1 (w then w-a).
        # f_0 = 1 - f_-1 - f_+1 is folded into e6 + fy/fx bias columns.
        # Partitions 0-7 = x of groups 0-7, 8-15 = y.
        fct = pmf.tile([16, 2, NPIX], BF16)
        FCH = 2048              # chunked so the m build can overlap
        for q0 in range(0, NPIX, FCH):
            f1 = fct[:, 0, q0:q0 + FCH]
            f2 = fct[:, 1, q0:q0 + FCH]
            oq = offs[:, q0:q0 + FCH]
            nc.vector.tensor_scalar(out=f1, in0=oq, scalar1=0.0,
                                    scalar2=0.0, op0=OP.is_lt, op1=OP.add)
            nc.vector.tensor_tensor(out=f2, in0=oq, in1=f1, op=OP.add)
            nc.vector.tensor_tensor(out=oq, in0=f1, in1=f2, op=OP.mult)
            nc.vector.tensor_tensor(out=f1, in0=f1, in1=oq, op=OP.subtract)
            nc.vector.tensor_tensor(out=f2, in0=f2, in1=oq, op=OP.subtract)
        poffs_cm.__exit__(None, None, None)

        # ============ DCN: build m for the whole band, then apply ========
        # Taps 0-5 accumulate on DVE, taps 6-8 on Pool; combine writes bf16.
        xd = pxd.tile([C, BAND, W], BF16)
        m_all = pma.tile([72, NPIX], BF16)
        with tc.tile_pool(name="dcmt", bufs=3) as dcmt, \
                tc.tile_pool(name="dcmp", bufs=2, space="PSUM") as dcmp:
            for cs in range(0, NPIX, 512):
                pFY = dcmp.tile([72, 512], F32, tag="pFY")
                pFX = dcmp.tile([72, 512], F32, tag="pFX")
                _mm(nc, pFY[:], s_e6[:, 0 * 72:1 * 72], fct[:, 0, cs:cs + 512],
                    start=True, stop=False)
                _mm(nc, pFY[:], s_e6[:, 1 * 72:2 * 72], fct[:, 1, cs:cs + 512],
                    start=False, stop=True)
                _mm(nc, pFX[:], s_e6[:, 2 * 72:3 * 72], fct[:, 0, cs:cs + 512],
                    start=True, stop=False)
                _mm(nc, pFX[:], s_e6[:, 3 * 72:4 * 72], fct[:, 1, cs:cs + 512],
                    start=False, stop=True)
                mfy = dcmt.tile([72, 512], BF16, tag="mfy")
                mfx = dcmt.tile([72, 512], BF16, tag="mfx")
                nc.scalar.activation(out=mfy[:], in_=pFY[:],
                                     func=AF.Identity, bias=s_fyb[:], scale=1.0)
                nc.scalar.activation(out=mfx[:], in_=pFX[:],
                                     func=AF.Identity, bias=s_fxb[:], scale=1.0)
                nc.vector.tensor_tensor(out=m_all[:, cs:cs + 512], in0=mfy[:],
                                        in1=mfx[:], op=OP.mult)
        pmf_cm.__exit__(None, None, None)   # fct dead once m_all is built

        DCH = 1024
        DR = DCH // W  # 8 rows per chunk
        with tc.tile_pool(name="dcnt", bufs=2) as dcnt, \
                tc.tile_pool(name="dcnb", bufs=3) as dcnb, \
                tc.tile_pool(name="dcnp", bufs=2, space="PSUM") as dcnp:
            # Pool taps (6,7,8) first, reading the PSUM expansion directly
            # (Pool gains nothing from bf16); DVE taps get an ACT bf16 cast
            # so every DVE multiply/add runs in 2x mode.
            TAP_ORDER = [6, 7, 8, 0, 1, 2, 3, 4, 5]
            for c0 in range(0, NPIX, DCH):
                t0 = c0 // W
                dstD = dcnt.tile([C, DR, W], BF16, tag="dstD")
                dstP = dcnt.tile([C, DR, W], BF16, tag="dstP")
                for k, ti in enumerate(TAP_ORDER):
                    dy, dx = ti // 3, ti % 3
                    pMB = dcnp.tile([C, DCH], F32, tag="pMB")
                    for s0 in range(0, DCH, 512):
                        _mm(nc, pMB[:, s0:s0 + 512], s_e16[:, ti * C:(ti + 1) * C],
                            m_all[:, c0 + s0:c0 + s0 + 512])
                    src = xc_pad[:, 1 + dy + t0:1 + dy + t0 + DR, dx:dx + W]
                    pmb_sb = dcnb.tile([C, DR, W], BF16,
                                       tag="pmbP" if k < 3 else "pmbD")
                    nc.scalar.activation(
                        out=pmb_sb[:],
                        in_=pMB[:].rearrange("p (a b) -> p a b", a=DR),
                        func=AF.Identity, bias=s_zero[:], scale=1.0)
                    eng = nc.gpsimd if k < 3 else nc.vector
                    acc = dstP if k < 3 else dstD
                    if k == 0 or k == 3:
                        eng.tensor_tensor(out=acc[:], in0=src,
                                          in1=pmb_sb[:], op=OP.mult)
                    else:
                        tmp = dcnb.tile([C, DR, W], BF16,
                                        tag="tmpP" if k < 3 else "tmpD")
                        eng.tensor_tensor(out=tmp[:], in0=src,
                                          in1=pmb_sb[:], op=OP.mult)
                        eng.tensor_tensor(out=acc[:], in0=acc[:],
                                          in1=tmp[:], op=OP.add)
                nc.vector.tensor_tensor(out=xd[:, t0:t0 + DR, :], in0=dstD[:],
                                        in1=dstP[:], op=OP.add)
        pma_cm.__exit__(None, None, None)
        pxc_cm.__exit__(None, None, None)

        # ====== scan prep: dA = sigmoid(-(dts+dt_b)), lndA = ln(dA) ======
        # delta = softplus(z) = -ln(dA); A = -1 (D_STATE=1) so
        # dA_scan = exp(-delta) = sigmoid(-z) and u = lndA * x * (-B).
        xdf = xd[:].rearrange("p a b -> p (a b)")
        pbig_cm = tc.tile_pool(name="pbig", bufs=1)
        pbig = pbig_cm.__enter__()
        lndA = pbig.tile([C, NPIX], BF16, tag="lndA")
        dA = pbig.tile([C, NPIX], F32, tag="dA")
        u = pbig.tile([C, NPIX], F32, tag="u")
        with tc.tile_pool(name="dtt", bufs=2) as dtt, \
                tc.tile_pool(name="pp2", bufs=2, space="PSUM") as pp2:
            for c0 in range(0, NPIX, 512):       # dA = sigmoid(-(dts + dtb))
                pt = pp2.tile([C, 512], F32, tag="pdts")
                _mm(nc, pt[:], s_dtW[:], xdf[:, c0:c0 + 512])
                nc.scalar.activation(out=dA[:, c0:c0 + 512], in_=pt[:],
                                     func=AF.Sigmoid, bias=s_ndtb[:], scale=-1.0)
            for c0 in range(0, NPIX, 512):       # lndA = ln(dA) = -delta
                nc.scalar.activation(out=lndA[:, c0:c0 + 512],
                                     in_=dA[:, c0:c0 + 512],
                                     func=AF.Ln, bias=s_zero[:], scale=1.0)
            for c0 in range(0, NPIX, 512):       # u = lndA * x * (-B)
                pb = pp2.tile([C, 512], F32, tag="pb")
                _mm(nc, pb[:], s_pbWn[:], xdf[:, c0:c0 + 512])
                dx_ = dtt.tile([C, 512], BF16, tag="dx")
                nc.vector.tensor_tensor(out=dx_[:], in0=lndA[:, c0:c0 + 512],
                                        in1=xdf[:, c0:c0 + 512], op=OP.mult)
                nc.vector.tensor_tensor(out=u[:, c0:c0 + 512], in0=dx_[:],
                                        in1=pb[:], op=OP.mult)

        # ================= selective scan + carry ========================
        h = pbig.tile([C, NPIX], F32, tag="h")
        nc.vector.tensor_tensor_scan(out=h[:], data0=dA[:], data1=u[:],
                                     initial=0.0, op0=OP.mult, op1=OP.add)
        # exchange h_last within band pairs (overlapped with E/y0 below)
        hc = sing.tile([C, 1], F32)
        nc.vector.tensor_tensor(out=hc[:], in0=h[:, NPIX - 1:NPIX], in1=s_mc[:],
                                op=OP.mult)
        dramp_cm = tc.tile_pool(name="dramp", bufs=1, space="DRAM")
        dramp = dramp_cm.__enter__()
        cc_in = dramp.tile([C, 1], F32)
        cc_out = dramp.tile([C, 1], F32)
        nc.sync.dma_start(out=cc_in[:], in_=hc[:])
        nc.gpsimd.collective_compute(
            "AllReduce", OP.add,
            replica_groups=[[0, 1], [2, 3], [4, 5], [6, 7]],
            ins=[cc_in[:].opt()], outs=[cc_out[:].opt()])
        h_in = sing.tile([C, 1], F32)
        nc.sync.dma_start(out=h_in[:], in_=cc_out[:])
        nc.vector.tensor_tensor(out=h_in[:], in0=h_in[:], in1=s_mu[:], op=OP.mult)
        # E = cumprod(dA) on Pool, concurrent with the DVE h-scan above
        zcol = sing.tile([C, 1], F32, tag="zcol")
        nc.gpsimd.memset(zcol[:], 0.0)
        E = pbig.tile([C, NPIX], F32, tag="E")
        nc.vector.tensor_tensor_scan(out=E[:], data0=dA[:],
                                     data1=zcol[:].broadcast_to([C, NPIX]),
                                     initial=1.0, op0=OP.mult, op1=OP.add)

        # ====== y0 = h*Cs + Ds*x and epc = E*Cs (pre-collective) =========
        y = pbig.tile([C, NPIX], BF16, tag="y")
        epc = pbig.tile([C, NPIX], BF16, tag="lndA")
        with tc.tile_pool(name="pcc", bufs=2, space="PSUM") as pcc:
            for c0 in range(0, NPIX, 512):
                pt = pcc.tile([C, 512], F32, tag="pc")
                _mm(nc, pt[:], s_pcW[:], xdf[:, c0:c0 + 512])
                nc.vector.tensor_tensor(out=y[:, c0:c0 + 512],
                                        in0=h[:, c0:c0 + 512],
                                        in1=pt[:], op=OP.mult)
                nc.vector.tensor_tensor(out=epc[:, c0:c0 + 512],
                                        in0=E[:, c0:c0 + 512],
                                        in1=pt[:], op=OP.mult)
        nc.vector.scalar_tensor_tensor(out=y[:], in0=xdf, scalar=s_ds[:],
                                       in1=y[:], op0=OP.mult, op1=OP.add)
        # post-collective: y += epc * h_in
        nc.vector.scalar_tensor_tensor(out=y[:], in0=epc[:], scalar=h_in[:],
                                       in1=y[:], op0=OP.mult, op1=OP.add)
        dramp_cm.__exit__(None, None, None)

        # ================= LN2 + out_proj ================================
        LNC2 = 512
        with tc.tile_pool(name="ln2t", bufs=2) as lnt2, \
                tc.tile_pool(name="ln2p", bufs=2, space="PSUM") as lnp2, \
                tc.tile_pool(name="ln2o", bufs=2, space="PSUM") as lnpo:
            for c0 in range(0, NPIX, LNC2):
                yc = y[:, c0:c0 + LNC2]
                sq = lnt2.tile([C, LNC2], BF16, tag="sq2")
                nc.gpsimd.tensor_tensor(out=sq[:], in0=yc, in1=yc, op=OP.mult)
                pA = lnp2.tile([C, LNC2], F32, tag="pA2")
                pB = lnp2.tile([C, LNC2], F32, tag="pB2")
                for s0 in range(0, LNC2, 512):
                    _mm(nc, pA[:, s0:s0 + 512], s_onesb[:], yc[:, s0:s0 + 512])
                    _mm(nc, pB[:, s0:s0 + 512], s_onesb[:], sq[:, s0:s0 + 512],
                        stop=False)
                    _mm(nc, pB[:, s0:s0 + 512], s_epsT[:], s_onerow[:],
                        start=False)
                mu2 = lnt2.tile([C, LNC2], BF16, tag="mu22")
                nc.scalar.activation(out=mu2[:], in_=pA[:], func=AF.Square,
                                     bias=s_zero[:], scale=1.0)
                varq = lnt2.tile([C, LNC2], F32, tag="varq2")
                nc.vector.tensor_tensor(out=varq[:], in0=pB[:], in1=mu2[:],
                                        op=OP.subtract)
                vr = lnt2.tile([C, LNC2], F32, tag="vr2")
                nc.vector.reciprocal(out=vr[:], in_=varq[:])
                r = lnt2.tile([C, LNC2], BF16, tag="r2")
                nc.scalar.activation(out=r[:], in_=vr[:], func=AF.Sqrt,
                                     bias=s_zero[:], scale=1.0)
                yn = lnt2.tile([C, LNC2], BF16, tag="yn")
                nc.vector.tensor_tensor(out=yn[:], in0=yc, in1=pA[:],
                                        op=OP.subtract)
                nc.vector.tensor_tensor(out=yn[:], in0=yn[:], in1=r[:], op=OP.mult)
                pO = lnpo.tile([C, LNC2], F32, tag="pO")
                for s0 in range(0, LNC2, 512):
                    _mm(nc, pO[:, s0:s0 + 512], s_outw[:], yn[:, s0:s0 + 512])
                osb = lnt2.tile([C, LNC2], F32, tag="osb")
                nc.scalar.activation(out=osb[:], in_=pO[:],
                                     func=AF.Identity, bias=s_wb2[:], scale=1.0)
                r0 = c0 // W
                nc.sync.dma_start(
                    out=out_band[:, r0:r0 + LNC2 // W, :],
                    in_=osb[:].rearrange("p (a b) -> p a b", a=LNC2 // W))
        pbig_cm.__exit__(None, None, None)
        pxd_cm.__exit__(None, None, None)
        est.close()
    nc.finalize()
    return nc


_CACHE = {}


def _host_prep(inputs):
    """Build the per-core in_maps from the full inputs."""
    bf = ml_dtypes.bfloat16
    x = inputs["x"].astype(np.float32)
    in_proj_w = inputs["in_proj_w"].astype(np.float32)
    k1 = inputs["conv2d_w"].astype(np.float32)[:, 0]        # (C,3,3)
    w_s1 = np.zeros((C, 9 * C), np.float32)                 # lhsT per tap [c, o]
    for ti in range(9):
        dy, dx = ti // 3, ti % 3
        Wt = in_proj_w * k1[:, dy, dx][:, None]             # (o, c)
        w_s1[:, ti * C:(ti + 1) * C] = Wt.T
    dwk = inputs["dw_w"].astype(np.float32)[:, 0]           # (C,3,3)
    dw_diag = np.zeros((C, 9 * C), np.float32)
    ar = np.arange(C)
    for ti in range(9):
        dw_diag[ar, ti * C + ar] = dwk[:, ti // 3, ti % 3]
    perm = list(range(0, 16, 2)) + list(range(1, 16, 2))
    off_w_p = inputs["off_w"].astype(np.float32)[perm]      # (16, C)
    off_b_p = inputs["off_b"].astype(np.float32)[perm]
    # expand blocks: m row p = dy*24 + dx*8 + g; fct row k = axis*8 + g
    # FY = f_-1 -> dy=-1 rows, -f_-1 -> dy=0; f_+1 -> dy=+1, -f_+1 -> dy=0;
    # +1 at dy=0 rows via fybias.  Same for FX over dx via fxbias.
    e6 = np.zeros((16, 4 * 72), np.float32)
    fybias = np.zeros((72, 1), np.float32)
    fxbias = np.zeros((72, 1), np.float32)
    for g in range(8):
        for d in range(3):
            # FY (uses y rows: k = 8 + g)
            e6[8 + g, 0 * 72 + 0 * 24 + d * 8 + g] = 1.0   # f_-1 -> dy=-1
            e6[8 + g, 0 * 72 + 1 * 24 + d * 8 + g] = -1.0  # -f_-1 -> dy=0
            e6[8 + g, 1 * 72 + 2 * 24 + d * 8 + g] = 1.0   # f_+1 -> dy=+1
            e6[8 + g, 1 * 72 + 1 * 24 + d * 8 + g] = -1.0  # -f_+1 -> dy=0
            fybias[1 * 24 + d * 8 + g, 0] = 1.0            # ones -> dy=0
            # FX (uses x rows: k = g)
            e6[0 + g, 2 * 72 + d * 24 + 0 * 8 + g] = 1.0
            e6[0 + g, 2 * 72 + d * 24 + 1 * 8 + g] = -1.0
            e6[0 + g, 3 * 72 + d * 24 + 2 * 8 + g] = 1.0
            e6[0 + g, 3 * 72 + d * 24 + 1 * 8 + g] = -1.0
            fxbias[d * 24 + 1 * 8 + g, 0] = 1.0
    e16 = np.zeros((72, 9 * C), np.float32)
    for ti in range(9):
        for c in range(C):
            e16[ti * 8 + c // GC, ti * C + c] = 1.0
    x_proj = inputs["x_proj_w"].astype(np.float32)          # (10, C)
    dt_w = inputs["dt_w"].astype(np.float32)                # (C, 8)
    dtW = dt_w @ x_proj[0:8]                                # (C, C)
    pbWn = -np.tile(x_proj[8][:, None], (1, C))             # lhsT[k, o]
    pcW = np.tile(x_proj[9][:, None], (1, C))
    ln2_g = inputs["out_ln_g"].astype(np.float32)
    ln2_b = inputs["out_ln_b"].astype(np.float32)
    out_w = inputs["out_proj_w"].astype(np.float32)
    shared = dict(
        w_s1=w_s1.astype(bf),
        conv2d_b=inputs["conv2d_b"].astype(np.float32).reshape(C, 1),
        dw_diag=dw_diag.astype(bf),
        dw_b=inputs["dw_b"].astype(np.float32).reshape(C, 1),
        ln1_g=inputs["dw_ln_g"].astype(np.float32).reshape(C, 1),
        ln1_b=inputs["dw_ln_b"].astype(np.float32).reshape(C, 1),
        off_w_lhsT=np.ascontiguousarray(off_w_p.T).astype(bf),
        off_b_col=off_b_p.reshape(16, 1),
        e6=e6.astype(bf), fybias=fybias, fxbias=fxbias, e16=e16.astype(bf),
        dtW_lhsT=np.ascontiguousarray(dtW.T).astype(bf),
        pbWn_lhsT=pbWn.astype(bf), pcW_lhsT=pcW.astype(bf),
        ndt_b_col=(-inputs["dt_b"].astype(np.float32)).reshape(C, 1),
        ds_col=inputs["Ds"].astype(np.float32).reshape(C, 1),
        onesb=np.full((C, C), 1.0 / C, np.float32).astype(bf),
        out_w_lhsT=np.ascontiguousarray((out_w * ln2_g[None, :]).T).astype(bf),
        wb2_col=(out_w @ ln2_b).reshape(C, 1),
    )
    in_maps = []
    for core in range(8):
        b, half = core // 2, core % 2
        r0 = half * BAND
        xp = np.zeros((C, NRX, WP), np.float32)
        lo, hi = r0 - XH, r0 + BAND + XH
        slo, shi = max(lo, 0), min(hi, H)
        xp[:, slo - lo:shi - lo, 1:W + 1] = x[b, :, slo:shi, :]
        im = dict(shared)
        im["x_pad"] = xp.astype(bf)
        im["mask_contrib"] = np.full((C, 1), 1.0 - half, np.float32)
        im["mask_use"] = np.full((C, 1), float(half), np.float32)
        in_maps.append(im)
    return in_maps


def kernel(**inputs) -> np.ndarray:
    if "nc" not in _CACHE:
        _CACHE["nc"] = build_program()
    nc = _CACHE["nc"]
    in_maps = _host_prep(inputs)
    res = run_bass_kernel_spmd(nc, in_maps, core_ids=list(range(8)))
    out = np.zeros((B, C, H, W), np.float32)
    for core in range(8):
        b, half = core // 2, core % 2
        out[b, :, half * BAND:(half + 1) * BAND, :] = res.results[core]["out"]
    return out


if __name__ == "__main__":
    import jax
    with jax.default_device(jax.devices("cpu")[0]):
        import reference as R
        inp = {k: np.asarray(v) for k, v in R.setup_inputs().items()}
    got = kernel(**inp)
    ref = np.load("/root/problem/ref_out.npy")
    rel = np.linalg.norm(got - ref) / np.linalg.norm(ref)
    print("Relative error:", rel)
